# revision 1
# baseline (speedup 1.0000x reference)
"""DiT attention kernel for 8 trn2 NeuronCores (nn_DiTAttention_40303973105877).

Sharding: sequence-parallel. Cores 0-3 <- batch 0, cores 4-7 <- batch 1;
each core owns 576 tokens of its batch. Only collective: 4-rank AllGather
of rope'd K (dim-major bf16) and V (token-major bf16, ones-column per head
appended so the PV matmul's 65th output row is the softmax denominator).

All matmuls bf16 with fp32 PSUM accumulation. RMSNorm rsqrt is computed as
exp(-0.5*ln(mean+eps)) so every activation lives in the single
natural_log_exp_and_others ACT table set.
"""

import sys

sys.path.insert(0, "/opt/trn_rl_repo")

import numpy as np
import ml_dtypes

import concourse.bacc as bacc
import concourse.mybir as mybir
import concourse.tile as tile
from concourse.bass_utils import run_bass_kernel_spmd

F32 = mybir.dt.float32
BF16 = mybir.dt.bfloat16
AF = mybir.ActivationFunctionType
OP = mybir.AluOpType

B, S, DIN, DIM, H = 2, 2304, 1024, 1024, 16
HD = 64  # head dim
NC_ = 8  # cores
TOK = 576  # tokens per core
KCH = S // 128  # 18 gathered k-chunks
DCH = DIM // 128  # 8 dim chunks
RG = [[0, 1, 2, 3], [4, 5, 6, 7]]
BASE = 10000.0
EPS = 1e-6
VW = H * (HD + 1)  # 1040: V row width incl. ones col per head

_cache = {}
import os
KPHASE = int(os.environ.get("KPHASE", "4"))
KCORES = int(os.environ.get("KCORES", str(NC_)))
KREPS = int(os.environ.get("KREPS", "1"))


def _bf(x):
    return np.ascontiguousarray(np.asarray(x).astype(ml_dtypes.bfloat16))


def _build_tables(t0, height, width, scale_vec):
    pos = (t0 + np.arange(TOK)).astype(np.float64)
    rows = np.floor(pos / width)
    cols = np.mod(pos, width)
    j = np.arange(16)
    invf = 1.0 / (BASE ** (2.0 * j / 32.0))  # (16,)
    ang_r = invf[:, None] * rows[None, :]  # (16, 576)
    ang_c = invf[:, None] * cols[None, :]
    cos64 = np.empty((64, TOK))
    sin64 = np.empty((64, TOK))  # sign-folded
    sigma = np.empty(64, dtype=np.int64)
    for d in range(16):
        cos64[d] = np.cos(ang_r[d])
        sin64[d] = -np.sin(ang_r[d])
        sigma[d] = d + 16
        cos64[d + 16] = np.cos(ang_r[d])
        sin64[d + 16] = np.sin(ang_r[d])
        sigma[d + 16] = d
        cos64[d + 32] = np.cos(ang_c[d])
        sin64[d + 32] = -np.sin(ang_c[d])
        sigma[d + 32] = d + 48
        cos64[d + 48] = np.cos(ang_c[d])
        sin64[d + 48] = np.sin(ang_c[d])
        sigma[d + 48] = d + 32
    cosf = np.tile(cos64, (H, 1))  # (1024, 576)
    sinf = np.tile(sin64, (H, 1))
    dg = np.arange(DIM)
    sig_g = (dg // 64) * 64 + sigma[dg % 64]
    cosq = cosf * scale_vec[:, None]
    sinq = sinf * scale_vec[sig_g][:, None]
    return cosq, sinq


def _r_matrix():
    """(128, 128) unsigned rotate-half permutation, rot = R @ x."""
    R = np.zeros((128, 128), dtype=np.float64)
    for blk in range(4):
        for d in range(16):
            R[blk * 32 + d, blk * 32 + d + 16] = 1.0
            R[blk * 32 + d + 16, blk * 32 + d] = 1.0
    return R


def _split_n(total, limit=512):
    out = []
    s = 0
    while s < total:
        e = min(s + limit, total)
        out.append((s, e))
        s = e
    return out


def _build_program():
    nc = bacc.Bacc("TRN2", target_bir_lowering=False, debug=False, num_devices=KCORES)

    d = {}
    for name, shape, dt in [
        ("xt", [DIN, TOK], BF16),
        ("wqkv", [DIN, 3 * DIM], BF16),
        ("pw", [DIM, DIN], BF16),
        ("cosq", [DIM, TOK], BF16),
        ("sinq", [DIM, TOK], BF16),
        ("cosk", [DIM, TOK], BF16),
        ("sink", [DIM, TOK], BF16),
        ("rmat", [128, 128], BF16),
        ("ones128", [128, 1], BF16),
        ("ones1x128", [1, 128], BF16),
        ("qkbias", [128, 16], F32),
        ("projb", [128, 8], F32),
        ("consts", [1, 4], F32),
    ]:
        d[name] = nc.dram_tensor(name, shape, dt, kind="ExternalInput").ap()
    out_d = nc.dram_tensor("out", [DIN, TOK], F32, kind="ExternalOutput").ap()

    with tile.TileContext(nc) as tc:
        with (
            tc.tile_pool(name="perm", bufs=1) as perm,
            tc.tile_pool(name="dram", bufs=1, space="DRAM") as dram,
        ):
            for _rep in range(KREPS):
                # ---- small residents ----
                consts = perm.tile([1, 4], F32, tag="consts")
                nc.sync.dma_start(out=consts[:], in_=d["consts"][:])
                warmup = perm.tile([1, 4], F32, tag="warmup")
                nc.scalar.copy(out=warmup[:], in_=consts[:])
                nc.scalar.activation(warmup[:], consts[:], AF.Exp)
                nc.scalar.activation(warmup[:], consts[:], AF.Square)
                nc.scalar.activation(
                    warmup[0:1, 0:1], consts[0:1, 0:1], AF.Ln
                )

                rmat = perm.tile([128, 128], BF16, tag="rmat")
                ones128 = perm.tile([128, 1], BF16, tag="ones128")
                ones1x = perm.tile([1, 128], BF16, tag="ones1x")
                qkbias = perm.tile([128, 16], F32, tag="qkbias")
                projb = perm.tile([128, 8], F32, tag="projb")
                nc.sync.dma_start(out=rmat[:], in_=d["rmat"][:])
                nc.sync.dma_start(out=ones128[:], in_=d["ones128"][:])
                nc.sync.dma_start(out=ones1x[:], in_=d["ones1x128"][:])
                nc.sync.dma_start(out=qkbias[:], in_=d["qkbias"][:])
                nc.sync.dma_start(out=projb[:], in_=d["projb"][:])

                qrope = perm.tile([128, DCH, TOK], BF16, tag="qrope")
                attn = perm.tile([128, DCH, TOK], BF16, tag="attn")

                kb = dram.tile([DIM, TOK], BF16)
                vb = dram.tile([TOK, VW], BF16)
                kg_d = dram.tile([4 * DIM, TOK], BF16)
                vg_d = dram.tile([S, VW], BF16)

                with tc.tile_pool(name="mid", bufs=1) as mid:
                    qraw = mid.tile([128, DCH, TOK], BF16, tag="qraw")
                    kraw = mid.tile([128, DCH, TOK], BF16, tag="kraw")
                    vloc = mid.tile([128, 5, VW], BF16, tag="vloc")
                    krope = mid.tile([128, DCH, TOK], BF16, tag="krope")

                    # ---- phase 1a: QKV projections ----
                    with tc.tile_pool(name="w1", bufs=1) as w1:
                        xt = w1.tile([128, DCH, TOK], BF16, tag="xt")
                        wq = w1.tile([128, DCH, 3 * DIM], BF16, tag="wq")
                        for c in range(DCH):
                            r = slice(128 * c, 128 * (c + 1))
                            nc.sync.dma_start(out=xt[:, c, :], in_=d["xt"][r, :])
                            nc.sync.dma_start(out=wq[:, c, :], in_=d["wqkv"][r, :])

                        with tc.tile_pool(name="qkvps", bufs=3, space="PSUM") as qkvps:
                            for raw, woff, boff in [(qraw, 0, 0), (kraw, DIM, 8)]:
                                for c in range(DCH):
                                    ps = qkvps.tile([128, DIM], F32, tag="qkv")
                                    for dc in range(DCH):
                                        for s0, s1 in _split_n(TOK):
                                            nc.tensor.matmul(
                                                ps[:, s0:s1],
                                                wq[
                                                    :, dc,
                                                    woff + 128 * c : woff + 128 * (c + 1),
                                                ],
                                                xt[:, dc, s0:s1],
                                                start=(dc == 0),
                                                stop=(dc == DCH - 1),
                                            )
                                    nc.scalar.activation(
                                        raw[:, c, :], ps[:, :TOK], AF.Identity,
                                        bias=qkbias[:, boff + c : boff + c + 1],
                                    )
                            for tch in range(5):
                                p = 128 if tch < 4 else 64
                                ps = qkvps.tile([128, DIM], F32, tag="qkv")
                                for dc in range(DCH):
                                    for s0, s1 in _split_n(DIM):
                                        nc.tensor.matmul(
                                            ps[:p, s0:s1],
                                            xt[:, dc, 128 * tch : 128 * tch + p],
                                            wq[:, dc, 2 * DIM + s0 : 2 * DIM + s1],
                                            start=(dc == 0),
                                            stop=(dc == DCH - 1),
                                        )
                                vv = vloc[:, tch, :].rearrange(
                                    "p (h w) -> p h w", h=H
                                )
                                nc.scalar.activation(
                                    vv[:p, :, 0:64],
                                    ps[:p, :].rearrange("p (h w) -> p h w", h=H),
                                    AF.Identity,
                                )
                                nc.gpsimd.memset(vv[:p, :, 64:65], 1.0)

                    # ---- phase 1b: sumsq -> rs rows (hi/lo bf16) ----
                    rs_rows = []
                    with (
                        tc.tile_pool(name="sqp", bufs=3) as sqp,
                        tc.tile_pool(name="ssps", bufs=2, space="PSUM") as ssps,
                    ):
                        for t, (raw, use_ln8) in enumerate(
                            [(qraw, True), (kraw, False)]
                        ):
                            ssp = ssps.tile([1, TOK], F32, tag="ss")
                            for c in range(DCH):
                                sq = sqp.tile([128, TOK], BF16, tag="sq")
                                nc.scalar.activation(sq[:], raw[:, c, :], AF.Square)
                                for s0, s1 in _split_n(TOK):
                                    nc.tensor.matmul(
                                        ssp[:, s0:s1],
                                        ones128[:],
                                        sq[:, s0:s1],
                                        start=(c == 0),
                                        stop=(c == DCH - 1),
                                    )
                            lnr = sqp.tile([1, TOK], F32, tag="lnr")
                            nc.scalar.activation(
                                lnr[:], ssp[:], AF.Ln,
                                bias=consts[0:1, 0:1], scale=1.0 / DIM,
                            )
                            rs = sqp.tile([1, TOK], F32, tag="rs")
                            nc.scalar.activation(
                                rs[:], lnr[:], AF.Exp,
                                bias=consts[0:1, 1:2] if use_ln8 else 0.0,
                                scale=-0.5,
                            )
                            rhi = mid.tile([1, TOK], BF16, tag=f"rhi{t}")
                            rlo = mid.tile([1, TOK], BF16, tag=f"rlo{t}")
                            nc.vector.tensor_copy(out=rhi[:], in_=rs[:])
                            nc.vector.tensor_tensor(
                                out=rlo[:], in0=rs[:], in1=rhi[:], op=OP.subtract
                            )
                            rs_rows.append((rhi, rlo))

                    # ---- phase 1c: rs bcast, rot, rope ----
                    with (
                        tc.tile_pool(name="tabs", bufs=1) as tabs,
                        tc.tile_pool(name="qbp", bufs=1) as qbp,
                        tc.tile_pool(name="scp", bufs=3) as scp,
                        tc.tile_pool(name="rbps", bufs=2, space="PSUM") as rbps,
                        tc.tile_pool(name="rotps", bufs=2, space="PSUM") as rotps,
                    ):
                        tabt = {}
                        for nm in ["cosq", "sinq", "cosk", "sink"]:
                            tabt[nm] = tabs.tile(
                                [128, DCH, TOK], BF16, tag=nm, name=f"tab_{nm}"
                            )
                            for c in range(DCH):
                                nc.sync.dma_start(
                                    out=tabt[nm][:, c, :],
                                    in_=d[nm][128 * c : 128 * (c + 1), :],
                                )
                        for t, (raw, rope, cosT, sinT) in enumerate(
                            [
                                (qraw, qrope, tabt["cosq"], tabt["sinq"]),
                                (kraw, krope, tabt["cosk"], tabt["sink"]),
                            ]
                        ):
                            rhi, rlo = rs_rows[t]
                            rbc = rbps.tile([128, TOK], F32, tag="rbc")
                            for s0, s1 in _split_n(TOK):
                                nc.tensor.matmul(
                                    rbc[:, s0:s1], ones1x[:], rhi[:, s0:s1],
                                    start=True, stop=False,
                                )
                                nc.tensor.matmul(
                                    rbc[:, s0:s1], ones1x[:], rlo[:, s0:s1],
                                    start=False, stop=True,
                                )
                            qb = qbp.tile([128, DCH, TOK], BF16, tag=f"qb{t}")
                            for c in range(DCH):
                                nc.vector.tensor_tensor(
                                    out=qb[:, c, :], in0=raw[:, c, :],
                                    in1=rbc[:], op=OP.mult,
                                )
                            for c in range(DCH):
                                rot = rotps.tile([128, TOK], F32, tag="rot")
                                for s0, s1 in _split_n(TOK):
                                    nc.tensor.matmul(
                                        rot[:, s0:s1], rmat[:], qb[:, c, s0:s1],
                                        start=True, stop=True,
                                    )
                                t1 = scp.tile([128, TOK], F32, tag="t1")
                                nc.vector.tensor_tensor(
                                    out=t1[:], in0=qb[:, c, :],
                                    in1=cosT[:, c, :], op=OP.mult,
                                )
                                t2 = scp.tile([128, TOK], F32, tag="t2")
                                nc.vector.tensor_tensor(
                                    out=t2[:], in0=rot[:], in1=sinT[:, c, :],
                                    op=OP.mult,
                                )
                                nc.vector.tensor_tensor(
                                    out=rope[:, c, :], in0=t1[:], in1=t2[:],
                                    op=OP.add,
                                )

                    # ---- phase 1d: bounce local K/V ----
                    for c in range(DCH):
                        nc.sync.dma_start(
                            out=kb[128 * c : 128 * (c + 1), :], in_=krope[:, c, :]
                        )
                    for tch in range(5):
                        p = 128 if tch < 4 else 64
                        nc.sync.dma_start(
                            out=vb[128 * tch : 128 * tch + p, :],
                            in_=vloc[:p, tch, :],
                        )

                # ---- phase 1 debug dump ----
                if KPHASE <= 1:
                    with tc.tile_pool(name="dbg0", bufs=2) as dbg0:
                        for c in range(DCH):
                            dt_ = dbg0.tile([128, TOK], F32, tag="dt")
                            nc.vector.tensor_copy(out=dt_[:], in_=qrope[:, c, :])
                            nc.sync.dma_start(
                                out=out_d[128 * c : 128 * (c + 1), :], in_=dt_[:]
                            )

                # ---- AllGather K and V within the batch group ----
                if KPHASE >= 2:
                    nc.gpsimd.collective_compute(
                    "AllGather", OP.bypass, replica_groups=RG,
                        ins=[kb[:].opt()], outs=[kg_d[:].opt()],
                    )
                    nc.gpsimd.collective_compute(
                        "AllGather", OP.bypass, replica_groups=RG,
                        ins=[vb[:].opt()], outs=[vg_d[:].opt()],
                    )

                if KPHASE >= 2:
                  with tc.tile_pool(name="ph2", bufs=1) as ph2:
                    kg = ph2.tile([128, DCH, S], BF16, tag="kg")
                    vg = ph2.tile([128, KCH, VW], BF16, tag="vg")
                    pw = ph2.tile([128, DCH, DIN], BF16, tag="pw")
                    for c in range(DCH):
                        for r in range(4):
                            nc.sync.dma_start(
                                out=kg[:, c, TOK * r : TOK * (r + 1)],
                                in_=kg_d[
                                    DIM * r + 128 * c : DIM * r + 128 * (c + 1), :
                                ],
                            )
                    for j in range(KCH):
                        nc.sync.dma_start(
                            out=vg[:, j, :], in_=vg_d[128 * j : 128 * (j + 1), :]
                        )
                    for c in range(DCH):
                        nc.sync.dma_start(
                            out=pw[:, c, :],
                            in_=d["pw"][128 * c : 128 * (c + 1), :],
                        )

                    if KPHASE == 2:
                        with tc.tile_pool(name="dbg2", bufs=2) as dbg2:
                            for c in range(DCH):
                                dt_ = dbg2.tile([128, TOK], F32, tag="dt")
                                nc.vector.tensor_copy(
                                    out=dt_[:], in_=kg[:, c, 0:TOK]
                                )
                                nc.sync.dma_start(
                                    out=out_d[128 * c : 128 * (c + 1), :],
                                    in_=dt_[:],
                                )

                    # ---- phase 2: attention per head pair ----
                    with (
                        tc.tile_pool(name="scps", bufs=2, space="PSUM") as scps,
                        tc.tile_pool(name="pvps", bufs=1, space="PSUM") as pvps,
                        tc.tile_pool(name="pp", bufs=4) as pp,
                        tc.tile_pool(name="denp", bufs=4) as denp,
                    ):
                        for hp in range(DCH if KPHASE >= 4 else (1 if KPHASE == 3 else 0)):
                            acc = [
                                pvps.tile(
                                    [65, TOK], F32, tag=f"acc{i}",
                                    name=f"acc{i}_{hp}",
                                )
                                for i in range(2)
                            ]
                            for kc in range(KCH):
                                ks = slice(128 * kc, 128 * (kc + 1))
                                for i in range(2):
                                    h = 2 * hp + i
                                    pr = slice(64 * i, 64 * i + 64)
                                    sth = scps.tile(
                                        [128, 1024], F32, tag="st",
                                        name=f"st{hp}_{kc}_{i}",
                                    )
                                    nc.tensor.matmul(
                                        sth[:, 0:512], kg[pr, hp, ks],
                                        qrope[pr, hp, 0:512],
                                        start=True, stop=True,
                                        tile_position=(64 * i, 0),
                                    )
                                    nc.tensor.matmul(
                                        sth[:, 512:576], kg[pr, hp, ks],
                                        qrope[pr, hp, 512:576],
                                        start=True, stop=True,
                                        tile_position=(64 * i, 0),
                                    )
                                    pth = pp.tile(
                                        [128, 576], BF16, tag="pt", bufs=8,
                                        name=f"pt{hp}_{kc}_{i}",
                                    )
                                    nc.scalar.activation(
                                        pth[:], sth[:, 0:576], AF.Exp
                                    )
                                    nc.tensor.matmul(
                                        acc[i][:, 0:512],
                                        vg[:, kc, 65 * h : 65 * h + 65],
                                        pth[:, 0:512],
                                        start=(kc == 0), stop=(kc == KCH - 1),
                                    )
                                    nc.tensor.matmul(
                                        acc[i][:, 512:576],
                                        vg[:, kc, 65 * h : 65 * h + 65],
                                        pth[:, 512:576],
                                        start=(kc == 0), stop=(kc == KCH - 1),
                                    )
                            # denominators -> bf16 reciprocal rows -> K=1 bcast
                            bcA = scps.tile(
                                [128, 1024], F32, tag="st", name=f"bcA_{hp}"
                            )
                            with nc.allow_low_precision("bf16 denom bcast"):
                                for i in range(2):
                                    rA = denp.tile(
                                        [1, TOK], BF16, tag=f"rA{i}",
                                        name=f"rA{i}_{hp}",
                                    )
                                    nc.vector.reciprocal(
                                        out=rA[:], in_=acc[i][64:65, :]
                                    )
                                    nc.tensor.matmul(
                                        bcA[64 * i : 64 * i + 64, 0:512],
                                        ones1x[:, 0:64], rA[:, 0:512],
                                        start=True, stop=True,
                                        tile_position=(0, 64 * i),
                                    )
                                    nc.tensor.matmul(
                                        bcA[64 * i : 64 * i + 64, 512:576],
                                        ones1x[:, 0:64], rA[:, 512:576],
                                        start=True, stop=True,
                                        tile_position=(0, 64 * i),
                                    )
                            bcs = denp.tile(
                                [128, TOK], F32, tag="bcs", name=f"bcs_{hp}"
                            )
                            nc.vector.tensor_copy(out=bcs[:], in_=bcA[:, :TOK])
                            for i in range(2):
                                nc.vector.tensor_tensor(
                                    out=attn[64 * i : 64 * i + 64, hp, :],
                                    in0=acc[i][0:64, :],
                                    in1=bcs[64 * i : 64 * i + 64, :],
                                    op=OP.mult,
                                )

                    # ---- phase 3: output projection ----
                    with (
                        tc.tile_pool(name="ops", bufs=2, space="PSUM") as ops,
                        tc.tile_pool(name="osb", bufs=2) as osb,
                    ):
                        for c in range(DCH if KPHASE >= 3 else 0):
                            ps = ops.tile([128, TOK], F32, tag="o")
                            for dc in range(DCH):
                                for s0, s1 in _split_n(TOK):
                                    nc.tensor.matmul(
                                        ps[:, s0:s1],
                                        pw[:, dc, 128 * c : 128 * (c + 1)],
                                        attn[:, dc, s0:s1],
                                        start=(dc == 0),
                                        stop=(dc == DCH - 1),
                                    )
                            ot = osb.tile([128, TOK], F32, tag="ot")
                            nc.scalar.activation(
                                ot[:], ps[:], AF.Identity,
                                bias=projb[:, c : c + 1],
                            )
                            nc.sync.dma_start(
                                out=out_d[128 * c : 128 * (c + 1), :], in_=ot[:]
                            )

    nc.compile()
    return nc


def kernel(input, qkv_w, qkv_b, q_scale, k_scale, proj_w, proj_b, height, width):
    input = np.asarray(input, dtype=np.float32)
    qkv_w = np.asarray(qkv_w, dtype=np.float32)
    qkv_b = np.asarray(qkv_b, dtype=np.float32)
    q_scale = np.asarray(q_scale, dtype=np.float32)
    k_scale = np.asarray(k_scale, dtype=np.float32)
    proj_w = np.asarray(proj_w, dtype=np.float32)
    proj_b = np.asarray(proj_b, dtype=np.float32)
    height = int(height)
    width = int(width)

    if "nc" not in _cache:
        _cache["nc"] = _build_program()
    nc = _cache["nc"]

    wqkvT = _bf(qkv_w.T)
    pwT = _bf(proj_w.T)
    rmat = _bf(_r_matrix())
    ones128 = _bf(np.ones((128, 1)))
    ones1x = _bf(np.ones((1, 128)))
    qkbias = np.zeros((128, 16), dtype=np.float32)
    for c in range(DCH):
        qkbias[:, c] = qkv_b[128 * c : 128 * (c + 1)]
        qkbias[:, 8 + c] = qkv_b[DIM + 128 * c : DIM + 128 * (c + 1)]
    projb_eff = proj_b + proj_w @ qkv_b[2 * DIM : 3 * DIM]
    projb = np.ascontiguousarray(
        projb_eff.reshape(DCH, 128).T.astype(np.float32)
    )
    consts = np.array([[EPS, -np.log(8.0), 0.0, 0.0]], dtype=np.float32)

    in_maps = []
    for c in range(NC_):
        b, r = c // 4, c % 4
        t0 = r * TOK
        cq, sq_ = _build_tables(t0, height, width, q_scale.astype(np.float64))
        ck, sk = _build_tables(t0, height, width, k_scale.astype(np.float64))
        in_maps.append(
            {
                "xt": _bf(input[b, t0 : t0 + TOK, :].T),
                "wqkv": wqkvT,
                "pw": pwT,
                "cosq": _bf(cq),
                "sinq": _bf(sq_),
                "cosk": _bf(ck),
                "sink": _bf(sk),
                "rmat": rmat,
                "ones128": ones128,
                "ones1x128": ones1x,
                "qkbias": qkbias,
                "projb": projb,
                "consts": consts,
            }
        )

    res = run_bass_kernel_spmd(
        nc, in_maps[:KCORES], core_ids=list(range(KCORES))
    )

    out = np.zeros((B, S, DIN), dtype=np.float32)
    for c in range(KCORES):
        b, r = c // 4, c % 4
        t0 = r * TOK
        out[b, t0 : t0 + TOK, :] = res.results[c]["out"].T
    return out



# revision 6
# speedup vs baseline: 448.2442x; 448.2442x over previous
"""DiT attention kernel for 8 trn2 NeuronCores (nn_DiTAttention_40303973105877).

Sharding: sequence-parallel. Cores 0-3 <- batch 0, cores 4-7 <- batch 1;
each core owns 576 tokens of its batch. Only collective: 4-rank AllGather
of rope'd K (dim-major bf16) and V (token-major bf16, ones-column per head
appended so the PV matmul's 65th output row is the softmax denominator).

All matmuls bf16 with fp32 PSUM accumulation. RMSNorm rsqrt is computed as
exp(-0.5*ln(mean+eps)) so every activation lives in the single
natural_log_exp_and_others ACT table set.
"""

import sys

sys.path.insert(0, "/opt/trn_rl_repo")

import numpy as np
import ml_dtypes

import concourse.bacc as bacc
import concourse.mybir as mybir
import concourse.tile as tile
from concourse.bass_utils import run_bass_kernel_spmd

F32 = mybir.dt.float32
BF16 = mybir.dt.bfloat16
AF = mybir.ActivationFunctionType
OP = mybir.AluOpType

B, S, DIN, DIM, H = 2, 2304, 1024, 1024, 16
HD = 64  # head dim
NC_ = 8  # cores
TOK = 576  # tokens per core
KCH = S // 128  # 18 gathered k-chunks
DCH = DIM // 128  # 8 dim chunks
RG = [[0, 1, 2, 3], [4, 5, 6, 7]]
BASE = 10000.0
EPS = 1e-6
VW = H * (HD + 1)  # 1040: V row width incl. ones col per head

_cache = {}
import os
KPHASE = int(os.environ.get("KPHASE", "4"))
KCORES = int(os.environ.get("KCORES", str(NC_)))
KREPS = int(os.environ.get("KREPS", "1"))
KLOOP = int(os.environ.get("KLOOP", "0"))


def _rep_iter(tc):
    """KREPS repetitions: hardware For_i loop (KLOOP=1, constant NEFF size,
    but collectives cannot live inside control flow) or python unroll."""
    if KLOOP:
        with tc.For_i(0, KREPS):
            yield 0
    else:
        for r in range(KREPS):
            yield r


def _bf(x):
    return np.ascontiguousarray(np.asarray(x).astype(ml_dtypes.bfloat16))


def _build_tables(t0, height, width, scale_vec):
    pos = (t0 + np.arange(TOK)).astype(np.float64)
    rows = np.floor(pos / width)
    cols = np.mod(pos, width)
    j = np.arange(16)
    invf = 1.0 / (BASE ** (2.0 * j / 32.0))  # (16,)
    ang_r = invf[:, None] * rows[None, :]  # (16, 576)
    ang_c = invf[:, None] * cols[None, :]
    cos64 = np.empty((64, TOK))
    sin64 = np.empty((64, TOK))  # sign-folded
    sigma = np.empty(64, dtype=np.int64)
    for d in range(16):
        cos64[d] = np.cos(ang_r[d])
        sin64[d] = -np.sin(ang_r[d])
        sigma[d] = d + 16
        cos64[d + 16] = np.cos(ang_r[d])
        sin64[d + 16] = np.sin(ang_r[d])
        sigma[d + 16] = d
        cos64[d + 32] = np.cos(ang_c[d])
        sin64[d + 32] = -np.sin(ang_c[d])
        sigma[d + 32] = d + 48
        cos64[d + 48] = np.cos(ang_c[d])
        sin64[d + 48] = np.sin(ang_c[d])
        sigma[d + 48] = d + 32
    cosf = np.tile(cos64, (H, 1))  # (1024, 576)
    sinf = np.tile(sin64, (H, 1))
    dg = np.arange(DIM)
    sig_g = (dg // 64) * 64 + sigma[dg % 64]
    cosq = cosf * scale_vec[:, None]
    sinq = sinf * scale_vec[sig_g][:, None]
    return cosq, sinq


def _r_matrix():
    """(128, 128) unsigned rotate-half permutation, rot = R @ x."""
    R = np.zeros((128, 128), dtype=np.float64)
    for blk in range(4):
        for d in range(16):
            R[blk * 32 + d, blk * 32 + d + 16] = 1.0
            R[blk * 32 + d + 16, blk * 32 + d] = 1.0
    return R


def _split_n(total, limit=512):
    out = []
    s = 0
    while s < total:
        e = min(s + limit, total)
        out.append((s, e))
        s = e
    return out


def _build_program():
    nc = bacc.Bacc("TRN2", target_bir_lowering=False, debug=False, num_devices=KCORES)

    d = {}
    for name, shape, dt in [
        ("xt", [DIN, TOK], BF16),
        ("wqkv", [DIN, 3 * DIM], BF16),
        ("pw", [DIM, DIN], BF16),
        ("cosq", [DIM, TOK], BF16),
        ("sinq", [DIM, TOK], BF16),
        ("cosk", [DIM, TOK], BF16),
        ("sink", [DIM, TOK], BF16),
        ("rmat", [128, 128], BF16),
        ("ones128", [128, 1], BF16),
        ("ones1x128", [1, 128], BF16),
        ("qkbias", [128, 16], F32),
        ("projb", [128, 8], F32),
        ("consts", [1, 4], F32),
    ]:
        d[name] = nc.dram_tensor(name, shape, dt, kind="ExternalInput").ap()
    out_d = nc.dram_tensor("out", [DIN, TOK], F32, kind="ExternalOutput").ap()

    with tile.TileContext(nc) as tc:
        with (
            tc.tile_pool(name="perm", bufs=1) as perm,
            tc.tile_pool(name="dram", bufs=1, space="DRAM") as dram,
        ):
            for _rep in _rep_iter(tc):
                # ---- small residents ----
                consts = perm.tile([1, 4], F32, tag="consts")
                nc.sync.dma_start(out=consts[:], in_=d["consts"][:])
                warmup = perm.tile([1, 4], F32, tag="warmup")
                nc.scalar.copy(out=warmup[:], in_=consts[:])
                nc.scalar.activation(warmup[:], consts[:], AF.Exp)
                nc.scalar.activation(warmup[:], consts[:], AF.Square)
                nc.scalar.activation(
                    warmup[0:1, 0:1], consts[0:1, 0:1], AF.Ln
                )

                rmat = perm.tile([128, 128], BF16, tag="rmat")
                ones128 = perm.tile([128, 1], BF16, tag="ones128")
                ones1x = perm.tile([1, 128], BF16, tag="ones1x")
                qkbias = perm.tile([128, 16], F32, tag="qkbias")
                projb = perm.tile([128, 8], F32, tag="projb")
                nc.sync.dma_start(out=rmat[:], in_=d["rmat"][:])
                nc.sync.dma_start(out=ones128[:], in_=d["ones128"][:])
                nc.sync.dma_start(out=ones1x[:], in_=d["ones1x128"][:])
                nc.sync.dma_start(out=qkbias[:], in_=d["qkbias"][:])
                nc.sync.dma_start(out=projb[:], in_=d["projb"][:])

                qrope = perm.tile([128, DCH, TOK], BF16, tag="qrope")
                attn = perm.tile([128, DCH, TOK], BF16, tag="attn")

                kb = dram.tile([DIM, TOK], BF16)
                vb = dram.tile([TOK, VW], BF16)
                kg_d = dram.tile([4 * DIM, TOK], BF16)
                vg_d = dram.tile([S, VW], BF16)

                with tc.tile_pool(name="mid", bufs=1) as mid:
                    qraw = mid.tile([128, DCH, TOK], BF16, tag="qraw")
                    kraw = mid.tile([128, DCH, TOK], BF16, tag="kraw")
                    vloc = mid.tile([128, 5, VW], BF16, tag="vloc")
                    krope = mid.tile([128, DCH, TOK], BF16, tag="krope")

                    # ---- phase 1a: QKV projections ----
                    with tc.tile_pool(name="w1", bufs=1) as w1:
                        xt = w1.tile([128, DCH, TOK], BF16, tag="xt")
                        wq = w1.tile([128, DCH, 3 * DIM], BF16, tag="wq")
                        for c in range(DCH):
                            r = slice(128 * c, 128 * (c + 1))
                            nc.sync.dma_start(out=xt[:, c, :], in_=d["xt"][r, :])
                            nc.sync.dma_start(out=wq[:, c, :], in_=d["wqkv"][r, :])

                        with tc.tile_pool(name="qkvps", bufs=3, space="PSUM") as qkvps:
                            for raw, woff, boff in [(qraw, 0, 0), (kraw, DIM, 8)]:
                                for c in range(DCH):
                                    ps = qkvps.tile([128, DIM], F32, tag="qkv")
                                    for dc in range(DCH):
                                        for s0, s1 in _split_n(TOK):
                                            nc.tensor.matmul(
                                                ps[:, s0:s1],
                                                wq[
                                                    :, dc,
                                                    woff + 128 * c : woff + 128 * (c + 1),
                                                ],
                                                xt[:, dc, s0:s1],
                                                start=(dc == 0),
                                                stop=(dc == DCH - 1),
                                            )
                                    nc.scalar.activation(
                                        raw[:, c, :], ps[:, :TOK], AF.Identity,
                                        bias=qkbias[:, boff + c : boff + c + 1],
                                    )
                            for tch in range(5):
                                p = 128 if tch < 4 else 64
                                ps = qkvps.tile([128, DIM], F32, tag="qkv")
                                for dc in range(DCH):
                                    for s0, s1 in _split_n(DIM):
                                        nc.tensor.matmul(
                                            ps[:p, s0:s1],
                                            xt[:, dc, 128 * tch : 128 * tch + p],
                                            wq[:, dc, 2 * DIM + s0 : 2 * DIM + s1],
                                            start=(dc == 0),
                                            stop=(dc == DCH - 1),
                                        )
                                vv = vloc[:, tch, :].rearrange(
                                    "p (h w) -> p h w", h=H
                                )
                                nc.scalar.activation(
                                    vv[:p, :, 0:64],
                                    ps[:p, :].rearrange("p (h w) -> p h w", h=H),
                                    AF.Identity,
                                )
                                nc.gpsimd.memset(vv[:p, :, 64:65], 1.0)

                    # ---- phase 1b: sumsq -> rs rows (hi/lo bf16) ----
                    rs_rows = []
                    with (
                        tc.tile_pool(name="sqp", bufs=3) as sqp,
                        tc.tile_pool(name="ssps", bufs=2, space="PSUM") as ssps,
                    ):
                        for t, (raw, use_ln8) in enumerate(
                            [(qraw, True), (kraw, False)]
                        ):
                            ssp = ssps.tile([1, TOK], F32, tag="ss")
                            for c in range(DCH):
                                sq = sqp.tile([128, TOK], BF16, tag="sq")
                                nc.scalar.activation(sq[:], raw[:, c, :], AF.Square)
                                for s0, s1 in _split_n(TOK):
                                    nc.tensor.matmul(
                                        ssp[:, s0:s1],
                                        ones128[:],
                                        sq[:, s0:s1],
                                        start=(c == 0),
                                        stop=(c == DCH - 1),
                                    )
                            lnr = sqp.tile([1, TOK], F32, tag="lnr")
                            nc.scalar.activation(
                                lnr[:], ssp[:], AF.Ln,
                                bias=consts[0:1, 0:1], scale=1.0 / DIM,
                            )
                            rs = sqp.tile([1, TOK], F32, tag="rs")
                            nc.scalar.activation(
                                rs[:], lnr[:], AF.Exp,
                                bias=consts[0:1, 1:2] if use_ln8 else 0.0,
                                scale=-0.5,
                            )
                            rhi = mid.tile([1, TOK], BF16, tag=f"rhi{t}")
                            rlo = mid.tile([1, TOK], BF16, tag=f"rlo{t}")
                            nc.vector.tensor_copy(out=rhi[:], in_=rs[:])
                            nc.vector.tensor_tensor(
                                out=rlo[:], in0=rs[:], in1=rhi[:], op=OP.subtract
                            )
                            rs_rows.append((rhi, rlo))

                    # ---- phase 1c: rs bcast, rot, rope ----
                    with (
                        tc.tile_pool(name="tabs", bufs=1) as tabs,
                        tc.tile_pool(name="qbp", bufs=1) as qbp,
                        tc.tile_pool(name="scp", bufs=3) as scp,
                        tc.tile_pool(name="rbps", bufs=2, space="PSUM") as rbps,
                        tc.tile_pool(name="rotps", bufs=2, space="PSUM") as rotps,
                    ):
                        tabt = {}
                        for nm in ["cosq", "sinq", "cosk", "sink"]:
                            tabt[nm] = tabs.tile(
                                [128, DCH, TOK], BF16, tag=nm, name=f"tab_{nm}"
                            )
                            for c in range(DCH):
                                nc.sync.dma_start(
                                    out=tabt[nm][:, c, :],
                                    in_=d[nm][128 * c : 128 * (c + 1), :],
                                )
                        for t, (raw, rope, cosT, sinT) in enumerate(
                            [
                                (qraw, qrope, tabt["cosq"], tabt["sinq"]),
                                (kraw, krope, tabt["cosk"], tabt["sink"]),
                            ]
                        ):
                            rhi, rlo = rs_rows[t]
                            rbc = rbps.tile([128, TOK], F32, tag="rbc")
                            for s0, s1 in _split_n(TOK):
                                nc.tensor.matmul(
                                    rbc[:, s0:s1], ones1x[:], rhi[:, s0:s1],
                                    start=True, stop=False,
                                )
                                nc.tensor.matmul(
                                    rbc[:, s0:s1], ones1x[:], rlo[:, s0:s1],
                                    start=False, stop=True,
                                )
                            qb = qbp.tile([128, DCH, TOK], BF16, tag=f"qb{t}")
                            for c in range(DCH):
                                nc.vector.tensor_tensor(
                                    out=qb[:, c, :], in0=raw[:, c, :],
                                    in1=rbc[:], op=OP.mult,
                                )
                            for c in range(DCH):
                                rot = rotps.tile([128, TOK], F32, tag="rot")
                                for s0, s1 in _split_n(TOK):
                                    nc.tensor.matmul(
                                        rot[:, s0:s1], rmat[:], qb[:, c, s0:s1],
                                        start=True, stop=True,
                                    )
                                t1 = scp.tile([128, TOK], F32, tag="t1")
                                nc.vector.tensor_tensor(
                                    out=t1[:], in0=qb[:, c, :],
                                    in1=cosT[:, c, :], op=OP.mult,
                                )
                                t2 = scp.tile([128, TOK], F32, tag="t2")
                                nc.vector.tensor_tensor(
                                    out=t2[:], in0=rot[:], in1=sinT[:, c, :],
                                    op=OP.mult,
                                )
                                nc.vector.tensor_tensor(
                                    out=rope[:, c, :], in0=t1[:], in1=t2[:],
                                    op=OP.add,
                                )

                    # ---- phase 1d: bounce local K/V ----
                    for c in range(DCH):
                        nc.sync.dma_start(
                            out=kb[128 * c : 128 * (c + 1), :], in_=krope[:, c, :]
                        )
                    for tch in range(5):
                        p = 128 if tch < 4 else 64
                        nc.sync.dma_start(
                            out=vb[128 * tch : 128 * tch + p, :],
                            in_=vloc[:p, tch, :],
                        )

                # ---- phase 1 debug dump ----
                if KPHASE <= 1:
                    with tc.tile_pool(name="dbg0", bufs=2) as dbg0:
                        for c in range(DCH):
                            dt_ = dbg0.tile([128, TOK], F32, tag="dt")
                            nc.vector.tensor_copy(out=dt_[:], in_=qrope[:, c, :])
                            nc.sync.dma_start(
                                out=out_d[128 * c : 128 * (c + 1), :], in_=dt_[:]
                            )

                # ---- AllGather K and V within the batch group ----
                if KPHASE >= 2:
                    nc.gpsimd.collective_compute(
                    "AllGather", OP.bypass, replica_groups=RG,
                        ins=[kb[:].opt()], outs=[kg_d[:].opt()],
                    )
                    nc.gpsimd.collective_compute(
                        "AllGather", OP.bypass, replica_groups=RG,
                        ins=[vb[:].opt()], outs=[vg_d[:].opt()],
                    )

                if KPHASE >= 2:
                  with tc.tile_pool(name="ph2", bufs=1) as ph2:
                    kg = ph2.tile([128, DCH, S], BF16, tag="kg")
                    vg = ph2.tile([128, KCH, VW], BF16, tag="vg")
                    pw = ph2.tile([128, DCH, DIN], BF16, tag="pw")
                    for c in range(DCH):
                        for r in range(4):
                            nc.sync.dma_start(
                                out=kg[:, c, TOK * r : TOK * (r + 1)],
                                in_=kg_d[
                                    DIM * r + 128 * c : DIM * r + 128 * (c + 1), :
                                ],
                            )
                    for j in range(KCH):
                        nc.sync.dma_start(
                            out=vg[:, j, :], in_=vg_d[128 * j : 128 * (j + 1), :]
                        )
                    for c in range(DCH):
                        nc.sync.dma_start(
                            out=pw[:, c, :],
                            in_=d["pw"][128 * c : 128 * (c + 1), :],
                        )

                    if KPHASE == 2:
                        with tc.tile_pool(name="dbg2", bufs=2) as dbg2:
                            for c in range(DCH):
                                dt_ = dbg2.tile([128, TOK], F32, tag="dt")
                                nc.vector.tensor_copy(
                                    out=dt_[:], in_=kg[:, c, 0:TOK]
                                )
                                nc.sync.dma_start(
                                    out=out_d[128 * c : 128 * (c + 1), :],
                                    in_=dt_[:],
                                )

                    # ---- phase 2: attention per head pair ----
                    with (
                        tc.tile_pool(name="scps", bufs=2, space="PSUM") as scps,
                        tc.tile_pool(name="pvps", bufs=1, space="PSUM") as pvps,
                        tc.tile_pool(name="pp", bufs=4) as pp,
                        tc.tile_pool(name="denp", bufs=4) as denp,
                    ):
                        for hp in range(DCH if KPHASE >= 4 else (1 if KPHASE == 3 else 0)):
                            acc = [
                                pvps.tile(
                                    [65, TOK], F32, tag=f"acc{i}",
                                    name=f"acc{i}_{hp}",
                                )
                                for i in range(2)
                            ]
                            for kc in range(KCH):
                                ks = slice(128 * kc, 128 * (kc + 1))
                                for i in range(2):
                                    h = 2 * hp + i
                                    pr = slice(64 * i, 64 * i + 64)
                                    sth = scps.tile(
                                        [128, 1024], F32, tag="st",
                                        name=f"st{hp}_{kc}_{i}",
                                    )
                                    nc.tensor.matmul(
                                        sth[:, 0:512], kg[pr, hp, ks],
                                        qrope[pr, hp, 0:512],
                                        start=True, stop=True,
                                        tile_position=(64 * i, 0),
                                    )
                                    nc.tensor.matmul(
                                        sth[:, 512:576], kg[pr, hp, ks],
                                        qrope[pr, hp, 512:576],
                                        start=True, stop=True,
                                        tile_position=(64 * i, 0),
                                    )
                                    pth = pp.tile(
                                        [128, 576], BF16, tag="pt", bufs=8,
                                        name=f"pt{hp}_{kc}_{i}",
                                    )
                                    nc.scalar.activation(
                                        pth[:], sth[:, 0:576], AF.Exp
                                    )
                                    nc.tensor.matmul(
                                        acc[i][:, 0:512],
                                        vg[:, kc, 65 * h : 65 * h + 65],
                                        pth[:, 0:512],
                                        start=(kc == 0), stop=(kc == KCH - 1),
                                    )
                                    nc.tensor.matmul(
                                        acc[i][:, 512:576],
                                        vg[:, kc, 65 * h : 65 * h + 65],
                                        pth[:, 512:576],
                                        start=(kc == 0), stop=(kc == KCH - 1),
                                    )
                            # denominators -> bf16 reciprocal rows -> K=1 bcast
                            bcA = scps.tile(
                                [128, 1024], F32, tag="st", name=f"bcA_{hp}"
                            )
                            with nc.allow_low_precision("bf16 denom bcast"):
                                for i in range(2):
                                    rA = denp.tile(
                                        [1, TOK], BF16, tag=f"rA{i}",
                                        name=f"rA{i}_{hp}",
                                    )
                                    nc.vector.reciprocal(
                                        out=rA[:], in_=acc[i][64:65, :]
                                    )
                                    nc.tensor.matmul(
                                        bcA[64 * i : 64 * i + 64, 0:512],
                                        ones1x[:, 0:64], rA[:, 0:512],
                                        start=True, stop=True,
                                        tile_position=(0, 64 * i),
                                    )
                                    nc.tensor.matmul(
                                        bcA[64 * i : 64 * i + 64, 512:576],
                                        ones1x[:, 0:64], rA[:, 512:576],
                                        start=True, stop=True,
                                        tile_position=(0, 64 * i),
                                    )
                            bcs = denp.tile(
                                [128, TOK], F32, tag="bcs", name=f"bcs_{hp}"
                            )
                            nc.vector.tensor_copy(out=bcs[:], in_=bcA[:, :TOK])
                            for i in range(2):
                                nc.vector.tensor_tensor(
                                    out=attn[64 * i : 64 * i + 64, hp, :],
                                    in0=acc[i][0:64, :],
                                    in1=bcs[64 * i : 64 * i + 64, :],
                                    op=OP.mult,
                                )

                    # ---- phase 3: output projection ----
                    with (
                        tc.tile_pool(name="ops", bufs=2, space="PSUM") as ops,
                        tc.tile_pool(name="osb", bufs=2) as osb,
                    ):
                        for c in range(DCH if KPHASE >= 3 else 0):
                            ps = ops.tile([128, TOK], F32, tag="o")
                            for dc in range(DCH):
                                for s0, s1 in _split_n(TOK):
                                    nc.tensor.matmul(
                                        ps[:, s0:s1],
                                        pw[:, dc, 128 * c : 128 * (c + 1)],
                                        attn[:, dc, s0:s1],
                                        start=(dc == 0),
                                        stop=(dc == DCH - 1),
                                    )
                            ot = osb.tile([128, TOK], F32, tag="ot")
                            nc.scalar.activation(
                                ot[:], ps[:], AF.Identity,
                                bias=projb[:, c : c + 1],
                            )
                            nc.sync.dma_start(
                                out=out_d[128 * c : 128 * (c + 1), :], in_=ot[:]
                            )

    nc.compile()
    return nc


def _prepare_in_maps(input, qkv_w, qkv_b, q_scale, k_scale, proj_w, proj_b,
                     height, width):
    input = np.asarray(input, dtype=np.float32)
    qkv_w = np.asarray(qkv_w, dtype=np.float32)
    qkv_b = np.asarray(qkv_b, dtype=np.float32)
    q_scale = np.asarray(q_scale, dtype=np.float32)
    k_scale = np.asarray(k_scale, dtype=np.float32)
    proj_w = np.asarray(proj_w, dtype=np.float32)
    proj_b = np.asarray(proj_b, dtype=np.float32)
    height = int(height)
    width = int(width)

    wqkvT = _bf(qkv_w.T)
    pwT = _bf(proj_w.T)
    rmat = _bf(_r_matrix())
    ones128 = _bf(np.ones((128, 1)))
    ones1x = _bf(np.ones((1, 128)))
    qkbias = np.zeros((128, 16), dtype=np.float32)
    for c in range(DCH):
        qkbias[:, c] = qkv_b[128 * c : 128 * (c + 1)]
        qkbias[:, 8 + c] = qkv_b[DIM + 128 * c : DIM + 128 * (c + 1)]
    projb_eff = proj_b + proj_w @ qkv_b[2 * DIM : 3 * DIM]
    projb = np.ascontiguousarray(
        projb_eff.reshape(DCH, 128).T.astype(np.float32)
    )
    consts = np.array([[EPS, -np.log(8.0), 0.0, 0.0]], dtype=np.float32)

    in_maps = []
    for c in range(NC_):
        b, r = c // 4, c % 4
        t0 = r * TOK
        cq, sq_ = _build_tables(t0, height, width, q_scale.astype(np.float64))
        ck, sk = _build_tables(t0, height, width, k_scale.astype(np.float64))
        in_maps.append(
            {
                "xt": _bf(input[b, t0 : t0 + TOK, :].T),
                "wqkv": wqkvT,
                "pw": pwT,
                "cosq": _bf(cq),
                "sinq": _bf(sq_),
                "cosk": _bf(ck),
                "sink": _bf(sk),
                "rmat": rmat,
                "ones128": ones128,
                "ones1x128": ones1x,
                "qkbias": qkbias,
                "projb": projb,
                "consts": consts,
            }
        )
    return in_maps


def _unshard_out(per_core_outs):
    out = np.zeros((B, S, DIN), dtype=np.float32)
    for c in range(KCORES):
        b, r = c // 4, c % 4
        t0 = r * TOK
        out[b, t0 : t0 + TOK, :] = per_core_outs[c].T
    return out


def kernel(input, qkv_w, qkv_b, q_scale, k_scale, proj_w, proj_b, height, width):
    if "nc" not in _cache:
        _cache["nc"] = _build_program()
    nc = _cache["nc"]

    in_maps = _prepare_in_maps(
        input, qkv_w, qkv_b, q_scale, k_scale, proj_w, proj_b, height, width
    )

    res = run_bass_kernel_spmd(
        nc, in_maps[:KCORES], core_ids=list(range(KCORES))
    )

    return _unshard_out([res.results[c]["out"] for c in range(KCORES)])

